# revision 13
# baseline (speedup 1.0000x reference)
"""Trainium2 Bass kernel for nn_AbsoluteHeadProbEncoder.

Math (mask all-ones, STEP=1, DAMP=0, REG=1):
  qz = x
  repeat 4x:
    S  = softmax(qz, axis=-1)                      # [L, d]
    W1T_c = T_c^T-contraction: W1T[b,i] = sum_a T[a,b,c] S[i,a]
    M_c[i,j] = sum_b W1T[b,i] S[j,b]  (logits = 64*M, diag -> -inf)
    E_c = softmax rows of 64*M_c (diag excluded), normalized
    P[j,(c,a)] = sum_b S[j,b] T[a,b,c] ; R[j,(c,a)] = sum_b S[j,b] T[b,a,c]
    T1[i,a] = sum_c sum_j E_c[i,j] P[j,(c,a)]
    T2[i,a] = sum_c sum_j E_c[j,i] R[j,(c,a)]
    qz = x + T1 + T2

Sharding: data-parallel over batch z (B=4) on cores 0-3; cores 4-7 run
duplicate batches (same SPMD program), outputs taken from cores 0-3.

Precision: matmul chain in float32r (TF32-class PE mode, ~1.7e-4 matmul
rel err measured), E in bf16, P in bf16 hi+lo split (stacked along the
weight M dim), R in bf16. Validated ~6e-3 max-rel-err vs fp64 reference.
"""
import sys
import numpy as np

if '/opt/trn_rl_repo' not in sys.path:
    sys.path.insert(0, '/opt/trn_rl_repo')

import concourse.bass as bass
import concourse.tile as tile
from concourse import mybir
from concourse.bass_utils import run_bass_kernel_spmd

B, L, D, H, NITER = 4, 512, 64, 8, 4
NB = L // 128            # 4 i/j blocks
NEG = 1e9

# blob layout (fp32 [128, 1920]):
#   x[0:256] | ident[256:384] | TCS[384:896] | TPS[896:1408] | TRS[1408:1920]
# TCS/TPS/TRS are bf16 hi/lo stacks: rows 0:64 = hi, rows 64:128 = lo.
XC0, IC0, TC0, TP0, TR0 = 0, 256, 384, 896, 1408
BLOBW = 1920

_SKIP_FIX = None


def _fix_waits(nc, max_inline=1):
    """Hoist excess per-instruction sem waits into standalone event-sem
    instructions (walrus encodes limited sync-wait slots per instruction)."""
    global _SKIP_FIX
    if _SKIP_FIX is None:
        _SKIP_FIX = (
            mybir.InstEventSemaphore, mybir.InstAllEngineBarrier,
            mybir.InstUnconditionalBranch, mybir.InstCompareAndBranch,
            mybir.InstIndirectBranch, mybir.InstBranchHint, mybir.InstHalt,
        )
    n = 0
    cnt = [0]
    for f in nc.m.functions:
        for bb in f.blocks:
            out = []
            for ins in bb.instructions:
                si = ins.sync_info
                if (si is not None and si.on_wait and len(si.on_wait) > max_inline
                        and not isinstance(ins, _SKIP_FIX)):
                    waits = list(si.on_wait)
                    extra, keep = waits[:-max_inline], waits[-max_inline:]
                    for w in extra:
                        cnt[0] += 1
                        ev = mybir.InstEventSemaphore(
                            name=f"I-waitfix-{cnt[0]}", ins=[], outs=[],
                            sync_info=mybir.SyncInfo(on_wait=[w], on_update=[]))
                        ev.engine = ins.engine
                        out.append(ev)
                    ins.sync_info = mybir.SyncInfo(
                        on_wait=keep, on_update=list(si.on_update or []))
                    n += 1
                out.append(ins)
            bb.instructions = out
    return n


def build_nc():
    f32 = mybir.dt.float32
    bf16 = mybir.dt.bfloat16
    f16 = mybir.dt.float16
    AF = mybir.ActivationFunctionType
    AX = mybir.AxisListType
    OP = mybir.AluOpType

    nc = bass.Bass()
    blob_ext = nc.declare_dram_parameter("blob", [128, BLOBW], f32, isOutput=False)
    out_ext = nc.declare_dram_parameter("out", [128, NB, D], f32, isOutput=True)

    with tile.TileContext(nc) as tc:
        with tc.tile_pool(name="const", bufs=1) as const, \
             tc.tile_pool(name="qzp", bufs=2) as qzp, \
             tc.tile_pool(name="smp", bufs=2) as smp, \
             tc.tile_pool(name="stp", bufs=2) as stp, \
             tc.tile_pool(name="w1p", bufs=8) as w1p, \
             tc.tile_pool(name="pwp", bufs=2) as pwp, \
             tc.tile_pool(name="ep", bufs=6) as ep, \
             tc.tile_pool(name="tiny", bufs=6) as tiny, \
             tc.tile_pool(name="ps_m", bufs=3, space="PSUM") as ps_m, \
             tc.tile_pool(name="ps_e", bufs=2, space="PSUM") as ps_e, \
             tc.tile_pool(name="ps_t", bufs=1, space="PSUM") as ps_t, \
             tc.tile_pool(name="ps_s", bufs=2, space="PSUM") as ps_s:

            blob = const.tile([128, BLOBW], f32)
            nc.sync.dma_start(out=blob, in_=blob_ext[:, :])
            x_sb = blob[:, XC0:XC0 + NB * D].rearrange("p (nb d) -> p nb d", nb=NB)
            ident = blob[:, IC0:IC0 + 128]

            # one-time const prep (bf16 casts; hi/lo values are exactly bf16)
            eye_bf = const.tile([128, 128], bf16)
            nc.vector.tensor_copy(eye_bf, ident)
            negeye = const.tile([128, 128], bf16)
            nc.vector.tensor_scalar_mul(negeye, ident, -NEG)
            eye_f16 = const.tile([128, 128], f16)
            nc.vector.tensor_copy(eye_f16, ident)
            tcs = const.tile([128, H * D], bf16)
            nc.vector.tensor_copy(tcs, blob[:, TC0:TC0 + H * D])
            tps = const.tile([128, H * D], bf16)
            nc.vector.tensor_copy(tps, blob[:, TP0:TP0 + H * D])
            trs = const.tile([128, H * D], bf16)
            nc.vector.tensor_copy(trs, blob[:, TR0:TR0 + H * D])

            qz_prev = None
            for it in range(NITER):
                # ---- softmax(qz) over d -> S [128, nb, 64] fp32
                src = x_sb if it == 0 else qz_prev
                negq = tiny.tile([128, NB], f32, tag="negq")
                nc.vector.tensor_reduce(negq, src, axis=AX.X, op=OP.max, negate=True)
                expq = smp.tile([128, NB, D], f32, tag="expq")
                rsq = tiny.tile([128, NB], f32, tag="rsq")
                for ib in range(NB):
                    nc.scalar.activation(expq[:, ib, :], src[:, ib, :], AF.Exp,
                                         bias=negq[:, ib:ib + 1], scale=1.0,
                                         accum_out=rsq[:, ib:ib + 1])
                rcq = tiny.tile([128, NB], f32, tag="rcq")
                nc.vector.reciprocal(rcq, rsq)
                s_sb = smp.tile([128, NB, D], f32, tag="s_sb")
                for ib in range(NB):
                    nc.vector.tensor_scalar_mul(s_sb[:, ib, :], expq[:, ib, :],
                                                rcq[:, ib:ib + 1])
                # hi/lo split of S, then transpose via identity matmuls
                shi = smp.tile([128, NB, D], bf16, tag="shi")
                nc.vector.tensor_copy(shi, s_sb)
                slo = smp.tile([128, NB, D], bf16, tag="slo")
                nc.vector.tensor_sub(slo, s_sb, shi)
                stps_hi = ps_s.tile([64, 512], f32, tag="ps_small")
                stps_lo = ps_s.tile([64, 512], f32, tag="ps_small")
                for ib in range(NB):
                    nc.tensor.matmul(stps_hi[:, 128 * ib:128 * (ib + 1)],
                                     shi[:, ib, :], eye_bf, start=True, stop=True,
                                     skip_group_check=True)
                    nc.tensor.matmul(stps_lo[:, 128 * ib:128 * (ib + 1)],
                                     slo[:, ib, :], eye_bf, start=True, stop=True,
                                     skip_group_check=True)
                st2 = stp.tile([128, 512], bf16, tag="st2")     # [SThi; STlo]
                nc.scalar.copy(st2[0:64, :], stps_hi)
                nc.scalar.copy(st2[64:128, :], stps_lo)
                st2hi = stp.tile([128, 512], bf16, tag="st2hi")  # [SThi; SThi]
                nc.vector.tensor_copy(st2hi[0:64, :], st2[0:64, :])
                nc.vector.tensor_copy(st2hi[64:128, :], st2[0:64, :])
                stlo0 = stp.tile([64, 512], bf16, tag="stlo0")   # STlo @ base 0
                nc.vector.tensor_copy(stlo0, st2[64:128, :])

                # ---- P/R production (hi/lo): PW [128, nb, H, 128], RW [...,D]
                pw = pwp.tile([128, NB, H, D + 1], f16, tag="pw")
                rw = pwp.tile([128, NB, H, D], f16, tag="rw")
                r_sb = pwp.tile([128, NB, H, D], f32, tag="r_sb")
                for jb in range(NB):
                    pps = ps_s.tile([128, 512], f32, tag="ps_small")
                    nc.tensor.matmul(pps, st2hi[:, 128 * jb:128 * (jb + 1)],
                                     tps, start=True, stop=False,
                                     skip_group_check=True)
                    nc.tensor.matmul(pps, stlo0[:, 128 * jb:128 * (jb + 1)],
                                     tps[0:64, :], start=False, stop=True,
                                     skip_group_check=True)
                    nc.scalar.copy(pw[:, jb, :, 0:D],
                                   pps.rearrange("p (c a) -> p c a", c=H))
                    nc.vector.memset(pw[:, jb, :, D:D + 1], 1.0)
                    rps = ps_s.tile([128, 512], f32, tag="ps_small")
                    nc.tensor.matmul(rps, st2hi[:, 128 * jb:128 * (jb + 1)],
                                     trs, start=True, stop=False,
                                     skip_group_check=True)
                    nc.tensor.matmul(rps, stlo0[:, 128 * jb:128 * (jb + 1)],
                                     trs[0:64, :], start=False, stop=True,
                                     skip_group_check=True)
                    nc.scalar.copy(r_sb[:, jb, :, :],
                                   rps.rearrange("p (c a) -> p c a", c=H))

                # ---- W1T for all heads: w1s[c] = [W1hi; W1lo] [128, 512] bf16
                w1list = []
                for c in range(H):
                    w1ps = ps_s.tile([64, 512], f32, tag="ps_small")
                    nc.tensor.matmul(w1ps, tcs[:, D * c:D * (c + 1)], st2hi,
                                     start=True, stop=False, skip_group_check=True)
                    nc.tensor.matmul(w1ps, tcs[0:64, D * c:D * (c + 1)],
                                     stlo0, start=False, stop=True,
                                     skip_group_check=True)
                    w1s = w1p.tile([128, 512], bf16, tag="w1s")
                    nc.scalar.copy(w1s[0:64, :], w1ps)
                    nc.vector.tensor_sub(w1s[64:128, :], w1ps, w1s[0:64, :])
                    w1list.append(w1s)

                # ---- per-head: M(+diag), rowmax, exp, normalize, ET, terms
                t2acc = ps_t.tile([64, 512], f32, tag="t2acc")
                acc1 = qzp.tile([128, NB, D], f32, tag="acc1")
                for c in range(H):
                    w1s = w1list[c]
                    e_raw = ep.tile([128, NB, 512], f16, tag="e_raw")
                    negm = tiny.tile([128, NB], f32, tag="negm")
                    for ib in range(NB):
                        mps = ps_m.tile([128, 512], f32, tag="mps")
                        nc.tensor.matmul(mps, w1s[:, 128 * ib:128 * (ib + 1)],
                                         st2hi, start=True, stop=False,
                                         skip_group_check=True)
                        nc.tensor.matmul(mps, w1s[0:64, 128 * ib:128 * (ib + 1)],
                                         stlo0, start=False, stop=False,
                                         skip_group_check=True)
                        nc.tensor.matmul(mps[:, 128 * ib:128 * (ib + 1)],
                                         negeye, eye_bf, start=False, stop=True,
                                         skip_group_check=True)
                        nc.vector.tensor_reduce(
                            negm[:, ib:ib + 1], mps,
                            axis=AX.X, op=OP.max, negate=True)
                        nc.scalar.activation(
                            e_raw[:, ib, :], mps,
                            AF.Exp, bias=negm[:, ib:ib + 1], scale=1.0)
                    # ET_raw[j, i] via regular matmuls against the identity
                    et = ep.tile([128, NB, 512], f16, tag="et")
                    for jb in range(NB):
                        etps = ps_e.tile([128, 512], f32, tag="etps")
                        for ib in range(NB):
                            nc.tensor.matmul(
                                etps[:, 128 * ib:128 * (ib + 1)],
                                e_raw[:, ib, 128 * jb:128 * (jb + 1)],
                                eye_f16, start=True, stop=True,
                                skip_group_check=True)
                        if jb % 2 == 0:
                            nc.scalar.copy(et[:, jb, :], etps)
                        else:
                            nc.vector.tensor_copy(et[:, jb, :], etps)
                    # Term1 per head (ones row -> exact quantized row-sums rs2)
                    t1c = ps_s.tile([D + 1, 512], f32, tag="ps_small")
                    for jb in range(NB):
                        nc.tensor.matmul(t1c, pw[:, jb, c, :], et[:, jb, :],
                                         start=(jb == 0), stop=(jb == NB - 1),
                                         skip_group_check=True)
                    t1sb = smp.tile([D + 1, 512], f32, tag="t1sbh")
                    nc.scalar.copy(t1sb, t1c)
                    t1t = ps_s.tile([128, 512], f32, tag="ps_small")
                    for ib in range(NB):
                        nc.tensor.transpose(t1t[:, 128 * ib:128 * ib + D + 1],
                                            t1sb[:, 128 * ib:128 * (ib + 1)],
                                            ident[0:D + 1, 0:D + 1])
                    t1t3 = t1t.rearrange("p (nb m) -> p nb m", nb=NB)
                    rc2 = tiny.tile([128, NB], f32, tag="rc2")
                    nc.vector.reciprocal(rc2, t1t3[:, :, D:D + 1].rearrange(
                        "p nb one -> p (nb one)"))
                    # accumulate normalized T1 into acc1
                    t1n = smp.tile([128, NB, D], f32, tag="t1n")
                    for ib in range(NB):
                        nc.vector.tensor_scalar_mul(t1n[:, ib, :], t1t3[:, ib, 0:D],
                                                    rc2[:, ib:ib + 1])
                    nc.vector.tensor_add(acc1, acc1 if c > 0 else x_sb, t1n)
                    # R' for this head, then Term2 accumulation
                    for jb in range(NB):
                        nc.vector.tensor_scalar_mul(rw[:, jb, c, :],
                                                    r_sb[:, jb, c, :],
                                                    rc2[:, jb:jb + 1])
                    first, last = (c == 0), (c == H - 1)
                    for jb in range(NB):
                        nc.tensor.matmul(t2acc, rw[:, jb, c, :], e_raw[:, jb, :],
                                         start=(first and jb == 0),
                                         stop=(last and jb == NB - 1),
                                         skip_group_check=True)

                # ---- combine: qz_new = x + T1hi + T1lo + T2
                t2sb = smp.tile([64, 512], f32, tag="t2sb")
                nc.scalar.copy(t2sb, t2acc)
                t2t = ps_s.tile([128, 512], f32, tag="ps_small")
                for ib in range(NB):
                    nc.tensor.transpose(t2t[:, 128 * ib:128 * ib + 64],
                                        t2sb[:, 128 * ib:128 * (ib + 1)],
                                        ident[0:64, 0:64])
                qz_new = qzp.tile([128, NB, D], f32, tag="qz")
                t2t3 = t2t.rearrange("p (nb m) -> p nb m", nb=NB)
                nc.vector.tensor_add(qz_new, acc1, t2t3[:, :, 0:64])
                qz_prev = qz_new

            nc.sync.dma_start(out=out_ext[:, :, :], in_=qz_prev)

    _fix_waits(nc)
    return nc


_NC_CACHE = None
_LAST_RESULTS = None


def _np_reference(x, mask, ternary):
    """Numpy fallback (general mask), used only if mask isn't all-ones."""
    O = dict(optimize=True)
    valid = (mask != 0)
    v1 = valid[:, :, None]
    pinv = ~(valid[:, None, :, None] & valid[:, None, None, :])
    diag = np.eye(L, dtype=np.float32) * NEG

    def sm(a):
        m = a.max(-1, keepdims=True)
        e = np.exp(a - m)
        return e / e.sum(-1, keepdims=True)

    qz = np.where(v1, x, 0.0).astype(np.float32)
    cn_qz = sm(qz)
    for it in range(NITER):
        nz = sm(qz)
        qz = nz
        qz = np.where(v1, qz, 0.0)
        msg_F = np.einsum('zia,zjb,abc->zcij', qz, qz, ternary, **O)
        qh = msg_F * D - diag
        qh = np.where(np.broadcast_to(pinv.transpose(0, 3, 1, 2), qh.shape), -NEG, qh)
        qh = sm(qh)
        G = (np.einsum('zjb,zcij,abc->zia', qz, qh, ternary, **O)
             + np.einsum('zjb,zcji,bac->zia', qz, qh, ternary, **O))
        qz = (x + G).astype(np.float32)
    return qz


def kernel(x, mask, ternary):
    x = np.ascontiguousarray(x, dtype=np.float32)
    ternary = np.ascontiguousarray(ternary, dtype=np.float32)
    if not np.all(np.asarray(mask) != 0):
        return _np_reference(x, np.asarray(mask), ternary)

    global _NC_CACHE
    if _NC_CACHE is None:
        _NC_CACHE = build_nc()
    nc = _NC_CACHE

    import ml_dtypes
    bfd = ml_dtypes.bfloat16

    def hilo(a):
        hi = a.astype(bfd).astype(np.float32)
        lo = (a - hi).astype(bfd).astype(np.float32)
        return np.concatenate([hi, lo], axis=0)   # [128, ...]

    ident = np.eye(128, dtype=np.float32)
    tc_host = np.transpose(ternary, (0, 2, 1)).reshape(64, H * D)      # [a,(c,b)]
    tp = np.transpose(ternary, (1, 2, 0)).reshape(64, H * D)           # [b,(c,a)]=T[a,b,c]
    tr = np.transpose(ternary, (0, 2, 1)).reshape(64, H * D)           # [b,(c,a)]=T[b,a,c]

    in_maps = []
    for core in range(8):
        z = core % B
        blob = np.zeros((128, BLOBW), np.float32)
        blob[:, XC0:XC0 + NB * D] = (
            x[z].reshape(NB, 128, D).transpose(1, 0, 2).reshape(128, NB * D))
        blob[:, IC0:IC0 + 128] = ident
        blob[:, TC0:TC0 + H * D] = hilo(tc_host) * float(D)
        blob[:, TP0:TP0 + H * D] = hilo(tp)
        blob[:, TR0:TR0 + H * D] = hilo(tr)
        in_maps.append({"blob": blob})

    global _LAST_RESULTS
    res = run_bass_kernel_spmd(nc, in_maps, core_ids=list(range(8)))
    _LAST_RESULTS = res
    out = np.empty((B, L, D), np.float32)
    for z in range(B):
        o = res.results[z]["out"]            # [128, NB, D]
        out[z] = o.transpose(1, 0, 2).reshape(L, D)
    return out


# revision 14
# speedup vs baseline: 1.1734x; 1.1734x over previous
"""Trainium2 Bass kernel for nn_AbsoluteHeadProbEncoder.

Math (mask all-ones, STEP=1, DAMP=0, REG=1):
  qz = x
  repeat 4x:
    S  = softmax(qz, axis=-1)                      # [L, d]
    W1T_c = T_c^T-contraction: W1T[b,i] = sum_a T[a,b,c] S[i,a]
    M_c[i,j] = sum_b W1T[b,i] S[j,b]  (logits = 64*M, diag -> -inf)
    E_c = softmax rows of 64*M_c (diag excluded), normalized
    P[j,(c,a)] = sum_b S[j,b] T[a,b,c] ; R[j,(c,a)] = sum_b S[j,b] T[b,a,c]
    T1[i,a] = sum_c sum_j E_c[i,j] P[j,(c,a)]
    T2[i,a] = sum_c sum_j E_c[j,i] R[j,(c,a)]
    qz = x + T1 + T2

Sharding: data-parallel over batch z (B=4) on cores 0-3; cores 4-7 run
duplicate batches (same SPMD program), outputs taken from cores 0-3.

Precision: matmul chain in float32r (TF32-class PE mode, ~1.7e-4 matmul
rel err measured), E in bf16, P in bf16 hi+lo split (stacked along the
weight M dim), R in bf16. Validated ~6e-3 max-rel-err vs fp64 reference.
"""
import sys
import numpy as np

if '/opt/trn_rl_repo' not in sys.path:
    sys.path.insert(0, '/opt/trn_rl_repo')

import concourse.bass as bass
import concourse.tile as tile
from concourse import mybir
from concourse.bass_utils import run_bass_kernel_spmd

B, L, D, H, NITER = 4, 512, 64, 8, 4
NB = L // 128            # 4 i/j blocks
NEG = 1e9

# blob layout (fp32 [128, 1920]):
#   x[0:256] | ident[256:384] | TCS[384:896] | TPS[896:1408] | TRS[1408:1920]
# TCS/TPS/TRS are bf16 hi/lo stacks: rows 0:64 = hi, rows 64:128 = lo.
XC0, IC0, TC0, TP0, TR0 = 0, 256, 384, 896, 1408
BLOBW = 1920

_SKIP_FIX = None


def _fix_waits(nc, max_inline=1):
    """Hoist excess per-instruction sem waits into standalone event-sem
    instructions (walrus encodes limited sync-wait slots per instruction)."""
    global _SKIP_FIX
    if _SKIP_FIX is None:
        _SKIP_FIX = (
            mybir.InstEventSemaphore, mybir.InstAllEngineBarrier,
            mybir.InstUnconditionalBranch, mybir.InstCompareAndBranch,
            mybir.InstIndirectBranch, mybir.InstBranchHint, mybir.InstHalt,
        )
    n = 0
    cnt = [0]
    for f in nc.m.functions:
        for bb in f.blocks:
            out = []
            for ins in bb.instructions:
                si = ins.sync_info
                if (si is not None and si.on_wait and len(si.on_wait) > max_inline
                        and not isinstance(ins, _SKIP_FIX)):
                    waits = list(si.on_wait)
                    extra, keep = waits[:-max_inline], waits[-max_inline:]
                    for w in extra:
                        cnt[0] += 1
                        ev = mybir.InstEventSemaphore(
                            name=f"I-waitfix-{cnt[0]}", ins=[], outs=[],
                            sync_info=mybir.SyncInfo(on_wait=[w], on_update=[]))
                        ev.engine = ins.engine
                        out.append(ev)
                    ins.sync_info = mybir.SyncInfo(
                        on_wait=keep, on_update=list(si.on_update or []))
                    n += 1
                out.append(ins)
            bb.instructions = out
    return n


def build_nc():
    f32 = mybir.dt.float32
    bf16 = mybir.dt.bfloat16
    f16 = mybir.dt.float16
    AF = mybir.ActivationFunctionType
    AX = mybir.AxisListType
    OP = mybir.AluOpType

    nc = bass.Bass()
    blob_ext = nc.declare_dram_parameter("blob", [128, BLOBW], f32, isOutput=False)
    out_ext = nc.declare_dram_parameter("out", [128, NB, D], f32, isOutput=True)

    with tile.TileContext(nc) as tc:
        with tc.tile_pool(name="const", bufs=1) as const, \
             tc.tile_pool(name="qzp", bufs=2) as qzp, \
             tc.tile_pool(name="smp", bufs=2) as smp, \
             tc.tile_pool(name="stp", bufs=2) as stp, \
             tc.tile_pool(name="w1p", bufs=8) as w1p, \
             tc.tile_pool(name="pwp", bufs=2) as pwp, \
             tc.tile_pool(name="ep", bufs=9) as ep, \
             tc.tile_pool(name="t1p", bufs=8) as t1p, \
             tc.tile_pool(name="tiny", bufs=6) as tiny, \
             tc.tile_pool(name="ps_m", bufs=3, space="PSUM") as ps_m, \
             tc.tile_pool(name="ps_e", bufs=2, space="PSUM") as ps_e, \
             tc.tile_pool(name="ps_t", bufs=1, space="PSUM") as ps_t, \
             tc.tile_pool(name="ps_s", bufs=2, space="PSUM") as ps_s:

            blob = const.tile([128, BLOBW], f32)
            nc.sync.dma_start(out=blob, in_=blob_ext[:, :])
            x_sb = blob[:, XC0:XC0 + NB * D].rearrange("p (nb d) -> p nb d", nb=NB)
            ident = blob[:, IC0:IC0 + 128]

            # one-time const prep (bf16 casts; hi/lo values are exactly bf16)
            eye_bf = const.tile([128, 128], bf16)
            nc.vector.tensor_copy(eye_bf, ident)
            negeye = const.tile([128, 128], bf16)
            nc.vector.tensor_scalar_mul(negeye, ident, -NEG)
            eye_f16 = const.tile([128, 128], f16)
            nc.vector.tensor_copy(eye_f16, ident)
            tcs = const.tile([128, H * D], bf16)
            nc.vector.tensor_copy(tcs, blob[:, TC0:TC0 + H * D])
            tps = const.tile([128, H * D], bf16)
            nc.vector.tensor_copy(tps, blob[:, TP0:TP0 + H * D])
            trs = const.tile([128, H * D], bf16)
            nc.vector.tensor_copy(trs, blob[:, TR0:TR0 + H * D])

            qz_prev = None
            for it in range(NITER):
                # ---- softmax(qz) over d -> S [128, nb, 64] fp32
                src = x_sb if it == 0 else qz_prev
                negq = tiny.tile([128, NB], f32, tag="negq")
                nc.vector.tensor_reduce(negq, src, axis=AX.X, op=OP.max, negate=True)
                expq = smp.tile([128, NB, D], f32, tag="expq")
                rsq = tiny.tile([128, NB], f32, tag="rsq")
                for ib in range(NB):
                    nc.scalar.activation(expq[:, ib, :], src[:, ib, :], AF.Exp,
                                         bias=negq[:, ib:ib + 1], scale=1.0,
                                         accum_out=rsq[:, ib:ib + 1])
                rcq = tiny.tile([128, NB], f32, tag="rcq")
                nc.vector.reciprocal(rcq, rsq)
                s_sb = smp.tile([128, NB, D], f32, tag="s_sb")
                for ib in range(NB):
                    nc.vector.tensor_scalar_mul(s_sb[:, ib, :], expq[:, ib, :],
                                                rcq[:, ib:ib + 1])
                # hi/lo split of S, then transpose via identity matmuls
                shi = smp.tile([128, NB, D], bf16, tag="shi")
                nc.vector.tensor_copy(shi, s_sb)
                slo = smp.tile([128, NB, D], bf16, tag="slo")
                nc.vector.tensor_sub(slo, s_sb, shi)
                stps_hi = ps_s.tile([64, 512], f32, tag="ps_small")
                stps_lo = ps_s.tile([64, 512], f32, tag="ps_small")
                for ib in range(NB):
                    nc.tensor.matmul(stps_hi[:, 128 * ib:128 * (ib + 1)],
                                     shi[:, ib, :], eye_bf, start=True, stop=True,
                                     skip_group_check=True)
                    nc.tensor.matmul(stps_lo[:, 128 * ib:128 * (ib + 1)],
                                     slo[:, ib, :], eye_bf, start=True, stop=True,
                                     skip_group_check=True)
                st2 = stp.tile([128, 512], bf16, tag="st2")     # [SThi; STlo]
                nc.scalar.copy(st2[0:64, :], stps_hi)
                nc.scalar.copy(st2[64:128, :], stps_lo)
                st2hi = stp.tile([128, 512], bf16, tag="st2hi")  # [SThi; SThi]
                nc.vector.tensor_copy(st2hi[0:64, :], st2[0:64, :])
                nc.vector.tensor_copy(st2hi[64:128, :], st2[0:64, :])
                stlo0 = stp.tile([64, 512], bf16, tag="stlo0")   # STlo @ base 0
                nc.vector.tensor_copy(stlo0, st2[64:128, :])

                # ---- P/R production (hi/lo): PW [128, nb, H, 128], RW [...,D]
                pw = pwp.tile([128, NB, H, D + 1], f16, tag="pw")
                rw = pwp.tile([128, NB, H, D], f16, tag="rw")
                r_sb = pwp.tile([128, NB, H, D], f32, tag="r_sb")
                for jb in range(NB):
                    pps = ps_s.tile([128, 512], f32, tag="ps_small")
                    nc.tensor.matmul(pps, st2hi[:, 128 * jb:128 * (jb + 1)],
                                     tps, start=True, stop=False,
                                     skip_group_check=True)
                    nc.tensor.matmul(pps, stlo0[:, 128 * jb:128 * (jb + 1)],
                                     tps[0:64, :], start=False, stop=True,
                                     skip_group_check=True)
                    nc.scalar.copy(pw[:, jb, :, 0:D],
                                   pps.rearrange("p (c a) -> p c a", c=H))
                    nc.vector.memset(pw[:, jb, :, D:D + 1], 1.0)
                    rps = ps_s.tile([128, 512], f32, tag="ps_small")
                    nc.tensor.matmul(rps, st2hi[:, 128 * jb:128 * (jb + 1)],
                                     trs, start=True, stop=False,
                                     skip_group_check=True)
                    nc.tensor.matmul(rps, stlo0[:, 128 * jb:128 * (jb + 1)],
                                     trs[0:64, :], start=False, stop=True,
                                     skip_group_check=True)
                    nc.scalar.copy(r_sb[:, jb, :, :],
                                   rps.rearrange("p (c a) -> p c a", c=H))

                # ---- W1T for all heads: w1s[c] = [W1hi; W1lo] [128, 512] bf16
                w1list = []
                for c in range(H):
                    w1ps = ps_s.tile([64, 512], f32, tag="ps_small")
                    nc.tensor.matmul(w1ps, tcs[:, D * c:D * (c + 1)], st2hi,
                                     start=True, stop=False, skip_group_check=True)
                    nc.tensor.matmul(w1ps, tcs[0:64, D * c:D * (c + 1)],
                                     stlo0, start=False, stop=True,
                                     skip_group_check=True)
                    w1s = w1p.tile([128, 512], bf16, tag="w1s")
                    nc.scalar.copy(w1s[0:64, :], w1ps)
                    nc.vector.tensor_sub(w1s[64:128, :], w1ps, w1s[0:64, :])
                    w1list.append(w1s)

                # ---- per-head: M(+diag), rowmax, exp, normalize, ET, terms
                t2acc = ps_t.tile([64, 512], f32, tag="t2acc")
                acc1 = qzp.tile([128, NB, D], f32, tag="acc1")
                er_list, t1_list = [], []
                for c in range(H):
                    w1s = w1list[c]
                    e_raw = ep.tile([128, NB, 512], f16, tag="e_raw")
                    negm = tiny.tile([128, NB], f32, tag="negm")
                    for ib in range(NB):
                        mps = ps_m.tile([128, 512], f32, tag="mps")
                        nc.tensor.matmul(mps, w1s[:, 128 * ib:128 * (ib + 1)],
                                         st2hi, start=True, stop=False,
                                         skip_group_check=True)
                        nc.tensor.matmul(mps, w1s[0:64, 128 * ib:128 * (ib + 1)],
                                         stlo0, start=False, stop=False,
                                         skip_group_check=True)
                        nc.tensor.matmul(mps[:, 128 * ib:128 * (ib + 1)],
                                         negeye, eye_bf, start=False, stop=True,
                                         skip_group_check=True)
                        nc.vector.tensor_reduce(
                            negm[:, ib:ib + 1], mps,
                            axis=AX.X, op=OP.max, negate=True)
                        nc.scalar.activation(
                            e_raw[:, ib, :], mps,
                            AF.Exp, bias=negm[:, ib:ib + 1], scale=1.0)
                    # ET_raw[j, i] via regular matmuls against the identity
                    et = ep.tile([128, NB, 512], f16, tag="et")
                    for jb in range(NB):
                        etps = ps_e.tile([128, 512], f32, tag="etps")
                        for ib in range(NB):
                            nc.tensor.matmul(
                                etps[:, 128 * ib:128 * (ib + 1)],
                                e_raw[:, ib, 128 * jb:128 * (jb + 1)],
                                eye_f16, start=True, stop=True,
                                skip_group_check=True)
                        if jb % 2 == 0:
                            nc.scalar.copy(et[:, jb, :], etps)
                        else:
                            nc.vector.tensor_copy(et[:, jb, :], etps)
                    # Term1 per head (ones row -> exact quantized row-sums rs2)
                    t1c = ps_s.tile([D + 1, 512], f32, tag="ps_small")
                    for jb in range(NB):
                        nc.tensor.matmul(t1c, pw[:, jb, c, :], et[:, jb, :],
                                         start=(jb == 0), stop=(jb == NB - 1),
                                         skip_group_check=True)
                    t1sb = t1p.tile([D + 1, 512], f32, tag="t1sbh")
                    nc.scalar.copy(t1sb, t1c)
                    er_list.append(e_raw)
                    t1_list.append(t1sb)
                for c in range(H):
                    e_raw, t1sb = er_list[c], t1_list[c]
                    t1t = ps_s.tile([128, 512], f32, tag="ps_small")
                    for ib in range(NB):
                        nc.tensor.transpose(t1t[:, 128 * ib:128 * ib + D + 1],
                                            t1sb[:, 128 * ib:128 * (ib + 1)],
                                            ident[0:D + 1, 0:D + 1])
                    t1t3 = t1t.rearrange("p (nb m) -> p nb m", nb=NB)
                    rc2 = tiny.tile([128, NB], f32, tag="rc2")
                    nc.vector.reciprocal(rc2, t1t3[:, :, D:D + 1].rearrange(
                        "p nb one -> p (nb one)"))
                    t1n = smp.tile([128, NB, D], f32, tag="t1n")
                    for ib in range(NB):
                        nc.vector.tensor_scalar_mul(t1n[:, ib, :], t1t3[:, ib, 0:D],
                                                    rc2[:, ib:ib + 1])
                    nc.vector.tensor_add(acc1, acc1 if c > 0 else x_sb, t1n)
                    for jb in range(NB):
                        nc.vector.tensor_scalar_mul(rw[:, jb, c, :],
                                                    r_sb[:, jb, c, :],
                                                    rc2[:, jb:jb + 1])
                    first, last = (c == 0), (c == H - 1)
                    for jb in range(NB):
                        nc.tensor.matmul(t2acc, rw[:, jb, c, :], e_raw[:, jb, :],
                                         start=(first and jb == 0),
                                         stop=(last and jb == NB - 1),
                                         skip_group_check=True)

                # ---- combine: qz_new = x + T1hi + T1lo + T2
                t2sb = smp.tile([64, 512], f32, tag="t2sb")
                nc.scalar.copy(t2sb, t2acc)
                t2t = ps_s.tile([128, 512], f32, tag="ps_small")
                for ib in range(NB):
                    nc.tensor.transpose(t2t[:, 128 * ib:128 * ib + 64],
                                        t2sb[:, 128 * ib:128 * (ib + 1)],
                                        ident[0:64, 0:64])
                qz_new = qzp.tile([128, NB, D], f32, tag="qz")
                t2t3 = t2t.rearrange("p (nb m) -> p nb m", nb=NB)
                nc.vector.tensor_add(qz_new, acc1, t2t3[:, :, 0:64])
                qz_prev = qz_new

            nc.sync.dma_start(out=out_ext[:, :, :], in_=qz_prev)

    _fix_waits(nc)
    return nc


_NC_CACHE = None
_LAST_RESULTS = None


def _np_reference(x, mask, ternary):
    """Numpy fallback (general mask), used only if mask isn't all-ones."""
    O = dict(optimize=True)
    valid = (mask != 0)
    v1 = valid[:, :, None]
    pinv = ~(valid[:, None, :, None] & valid[:, None, None, :])
    diag = np.eye(L, dtype=np.float32) * NEG

    def sm(a):
        m = a.max(-1, keepdims=True)
        e = np.exp(a - m)
        return e / e.sum(-1, keepdims=True)

    qz = np.where(v1, x, 0.0).astype(np.float32)
    cn_qz = sm(qz)
    for it in range(NITER):
        nz = sm(qz)
        qz = nz
        qz = np.where(v1, qz, 0.0)
        msg_F = np.einsum('zia,zjb,abc->zcij', qz, qz, ternary, **O)
        qh = msg_F * D - diag
        qh = np.where(np.broadcast_to(pinv.transpose(0, 3, 1, 2), qh.shape), -NEG, qh)
        qh = sm(qh)
        G = (np.einsum('zjb,zcij,abc->zia', qz, qh, ternary, **O)
             + np.einsum('zjb,zcji,bac->zia', qz, qh, ternary, **O))
        qz = (x + G).astype(np.float32)
    return qz


def kernel(x, mask, ternary):
    x = np.ascontiguousarray(x, dtype=np.float32)
    ternary = np.ascontiguousarray(ternary, dtype=np.float32)
    if not np.all(np.asarray(mask) != 0):
        return _np_reference(x, np.asarray(mask), ternary)

    global _NC_CACHE
    if _NC_CACHE is None:
        _NC_CACHE = build_nc()
    nc = _NC_CACHE

    import ml_dtypes
    bfd = ml_dtypes.bfloat16

    def hilo(a):
        hi = a.astype(bfd).astype(np.float32)
        lo = (a - hi).astype(bfd).astype(np.float32)
        return np.concatenate([hi, lo], axis=0)   # [128, ...]

    ident = np.eye(128, dtype=np.float32)
    tc_host = np.transpose(ternary, (0, 2, 1)).reshape(64, H * D)      # [a,(c,b)]
    tp = np.transpose(ternary, (1, 2, 0)).reshape(64, H * D)           # [b,(c,a)]=T[a,b,c]
    tr = np.transpose(ternary, (0, 2, 1)).reshape(64, H * D)           # [b,(c,a)]=T[b,a,c]

    in_maps = []
    for core in range(8):
        z = core % B
        blob = np.zeros((128, BLOBW), np.float32)
        blob[:, XC0:XC0 + NB * D] = (
            x[z].reshape(NB, 128, D).transpose(1, 0, 2).reshape(128, NB * D))
        blob[:, IC0:IC0 + 128] = ident
        blob[:, TC0:TC0 + H * D] = hilo(tc_host) * float(D)
        blob[:, TP0:TP0 + H * D] = hilo(tp)
        blob[:, TR0:TR0 + H * D] = hilo(tr)
        in_maps.append({"blob": blob})

    global _LAST_RESULTS
    res = run_bass_kernel_spmd(nc, in_maps, core_ids=list(range(8)))
    _LAST_RESULTS = res
    out = np.empty((B, L, D), np.float32)
    for z in range(B):
        o = res.results[z]["out"]            # [128, NB, D]
        out[z] = o.transpose(1, 0, 2).reshape(L, D)
    return out
